# revision 1
# baseline (speedup 1.0000x reference)
"""Cross-conditional GPT2 sparse attention block on 8 Trainium2 NeuronCores.

Sharding: core = (batch b in 0..3) x (head-group g in 0..1, 6 heads each).
Each core computes, for its (b, g):
  qT/kT = (Wq_g @ x_b^T + bq_g)  laid out [d_on_partitions, L]
  v     = x_b @ Wv_g^T + bv_g    natural layout [L, 384], interleaved with a
          ones column per head ([L, 6, 65]) so att@v also yields the softmax
          denominator for free.
  scores are computed *transposed* (sT[j, i]) so that softmax needs no
  transpose at all: exp on ACT, multiplicative 0/1 mask (host-built, bf16),
  att@v via lhsT=v (natural layout), denominator broadcast across partitions
  via a K=1 PE matmul, then the partial output projection with Wp[:, g]^T.
Host sums the two per-batch partials and adds bp.
"""

import sys

sys.path.insert(0, "/opt/trn_rl_repo")

from contextlib import ExitStack

import ml_dtypes
import numpy as np

import concourse.bacc as bacc
import concourse.bass as bass
import concourse.mybir as mybir
import concourse.tile as tile
from concourse.bass_utils import run_bass_kernel_spmd

# ---- problem constants (hardcoded per spec) ----
B = 4
T = 512
N = 8
C = 768
NHEAD = 12
L = 3 * T + 4 * N  # 1568
P = 128
G = C // 2  # 384 channels per head-group
NH = 6  # heads per core
D = 64  # head dim
ET = C // P  # 6 e-tiles (contraction of x @ W)
CT = G // P  # 3 c-tiles of the group's channels
NJT = (L + P - 1) // P  # 13 j tiles (12x128 + 32)
JPAD = NJT * P  # 1664
I_CHUNKS = [(0, 512), (512, 512), (1024, 512), (1536, 32)]
SCALE = 1.0 / 8.0  # 1/sqrt(64)

F32 = mybir.dt.float32
BF16 = mybir.dt.bfloat16
F16 = mybir.dt.float16

_NC = None  # cached compiled Bass program


def _jl(jt):
    return P if jt < NJT - 1 else L - (NJT - 1) * P  # 128 or 32


def _score_intervals(jt):
    """i-ranges (start, len) that can attend any column in j-tile jt.
    Derived from the cross-conditional mask block structure. The text-row
    strip [1536,1568) is merged into the preceding torso interval whenever
    the combined length fits one PSUM bank (<=512)."""
    if jt <= 3:
        j0 = jt * P
        iv = [(j0, 512 - j0), (512 + j0, 512 - j0), (1024 + j0, 512 - j0), (1536, 32)]
    elif jt <= 11:
        f0 = (jt % 4) * P
        iv = [(512 + f0, 512 - f0), (1024 + f0, 512 - f0), (1536, 32)]
    else:
        iv = [(512, 512), (1024, 512), (1536, 32)]
    if len(iv) >= 2 and iv[-2][0] + iv[-2][1] == 1536 and iv[-2][1] + 32 <= 512:
        iv = iv[:-2] + [(iv[-2][0], iv[-2][1] + 32)]
    return iv


def _ich_of(a):
    return 3 if a == 1536 else a // 512


_ATTV_LAST = {0: 3, 1: NJT - 1, 2: NJT - 1, 3: NJT - 1}  # last jt per ich

# (group) -> per-jt score interval (a, ln) and mask spec.
# g0 = upper rows (i 0..512), jts 0..3; g1 = lower rows; g2 = torso+text rows.
def _grp_interval(g, jt):
    j0 = jt * P
    f0 = (jt % 4) * P if jt <= 11 else 0
    if g == 0:
        return (j0, 512 - j0) if jt <= 3 else None
    if g == 1:
        s = j0 if jt <= 3 else f0
        return (512 + s, 512 - s)
    s = j0 if jt <= 3 else f0
    return (1024 + s, 544 - s)


# mask kind per (group, jt): 'T1' | 'T2' | 'TXT' | None
def _grp_mask(g, jt):
    if jt == 12:
        return "TXT" if g in (1, 2) else None
    if g == 0:
        return "T1"
    if g == 1:
        return "T1" if jt <= 3 else "T2"
    return "T1" if jt <= 7 else "T2"


_GRP_ITS = {0: range(0, 4), 1: range(4, 8), 2: range(8, 13)}



def _build_program():
    nc = bacc.Bacc("TRN2", target_bir_lowering=False, debug=False)

    xT_d = nc.dram_tensor("xT", [C, L], F16, kind="ExternalInput")
    wq_d = nc.dram_tensor("wqT", [C, G], F16, kind="ExternalInput")
    wk_d = nc.dram_tensor("wkT", [C, G], F16, kind="ExternalInput")
    wv_d = nc.dram_tensor("wvT", [C, G], F16, kind="ExternalInput")
    wp_d = nc.dram_tensor("wpT", [G, C], F16, kind="ExternalInput")
    bq_d = nc.dram_tensor("bqP", [P, CT], F32, kind="ExternalInput")
    bk_d = nc.dram_tensor("bkP", [P, CT], F32, kind="ExternalInput")
    bv_d = nc.dram_tensor("bvB", [P, G], F32, kind="ExternalInput")
    maskd_d = nc.dram_tensor("maskD", [P, 2, P], F16, kind="ExternalInput")
    maskt_d = nc.dram_tensor("maskTxt", [32, 1024], F16, kind="ExternalInput")
    out_d = nc.dram_tensor("out_part", [L, C], F32, kind="ExternalOutput")

    with tile.TileContext(nc) as tc, ExitStack() as big:
        persist = big.enter_context(tc.tile_pool(name="persist", bufs=1))

        # persistent SBUF tensors
        qT = persist.tile([P, CT, L], F16, name="qT")
        kT = persist.tile([P, CT, L], F16, name="kT")
        v_ones = persist.tile([P, NJT, NH, D + 1], F16, name="v_ones")
        maskD = persist.tile([P, 2, P], F16, name="maskD_sb")
        maskTx = persist.tile([32, 1024], F16, name="maskTx_sb")
        yT = persist.tile([P, CT, L], F16, name="yT")
        wp_sb = persist.tile([P, CT, C], F16, name="wp_sb")
        ones64 = persist.tile([1, D], F16, name="ones64")
        bv_sb = persist.tile([P, G], F32, name="bv_sb")

        nc.sync.dma_start(maskD[:], maskd_d[:])
        nc.sync.dma_start(maskTx[:], maskt_d[:])
        nc.sync.dma_start(wp_sb[:], wp_d.rearrange("(ct p) n -> p ct n", p=P))
        nc.sync.dma_start(bv_sb[:], bv_d[:])
        nc.gpsimd.memset(ones64[:], 1.0)
        nc.gpsimd.memset(v_ones[:], 1.0)

        # ---------- Phase A: projections ----------
        with (
            tc.tile_pool(name="phA", bufs=1) as phA,
            tc.tile_pool(name="psA", bufs=2, space="PSUM") as psA,
        ):
            xT = phA.tile([P, ET, L], F16, name="xT_sb")
            wq_sb = phA.tile([P, ET, G], F16, name="wq_sb")
            wk_sb = phA.tile([P, ET, G], F16, name="wk_sb")
            wv_sb = phA.tile([P, ET, G], F16, name="wv_sb")
            bq_sb = phA.tile([P, CT], F32, name="bq_sb")
            bk_sb = phA.tile([P, CT], F32, name="bk_sb")

            nc.sync.dma_start(xT[:], xT_d.rearrange("(et p) i -> p et i", p=P))
            nc.sync.dma_start(wq_sb[:], wq_d.rearrange("(et p) m -> p et m", p=P))
            nc.sync.dma_start(wk_sb[:], wk_d.rearrange("(et p) m -> p et m", p=P))
            nc.sync.dma_start(wv_sb[:], wv_d.rearrange("(et p) m -> p et m", p=P))
            nc.sync.dma_start(bq_sb[:], bq_d[:])
            nc.sync.dma_start(bk_sb[:], bk_d[:])

            # qT / kT: out[c_tile, i] accumulated over e tiles
            for dst, w_sb, b_sb in ((qT, wq_sb, bq_sb), (kT, wk_sb, bk_sb)):
                for ct in range(CT):
                    for i0, ilen in I_CHUNKS:
                        ps = psA.tile([P, 512], F32, name="ps_qk", tag="ps_qk")
                        for et in range(ET):
                            nc.tensor.matmul(
                                ps[:, :ilen],
                                w_sb[:, et, ct * P : (ct + 1) * P],
                                xT[:, et, i0 : i0 + ilen],
                                start=(et == 0),
                                stop=(et == ET - 1),
                            )
                        nc.vector.tensor_scalar(
                            dst[:, ct, i0 : i0 + ilen],
                            ps[:, :ilen],
                            b_sb[:, ct : ct + 1],
                            None,
                            mybir.AluOpType.add,
                        )

            # v natural layout [i, 384] + bias, into the 65-strided bf16 buffer
            for it in range(NJT):
                il = _jl(it)
                ps = psA.tile([P, G], F32, name="ps_v", tag="ps_v")
                for et in range(ET):
                    nc.tensor.matmul(
                        ps[:il, :],
                        xT[:, et, it * P : it * P + il],
                        wv_sb[:, et, :],
                        start=(et == 0),
                        stop=(et == ET - 1),
                    )
                nc.vector.tensor_tensor(
                    v_ones[:il, it, :, 0:D],
                    ps[:il, :].rearrange("p (h d) -> p h d", h=NH),
                    bv_sb[:il, :].rearrange("p (h d) -> p h d", h=NH),
                    mybir.AluOpType.add,
                )

        # ---------- Phase B+C: attention by row-group, proj interleaved ----------
        with (
            tc.tile_pool(name="phB", bufs=1) as phB,
            tc.tile_pool(name="phC", bufs=3) as phC,
            tc.tile_pool(name="psS", bufs=3, space="PSUM") as psS,
            tc.tile_pool(name="psY", bufs=5, space="PSUM") as psY,
        ):
            for g in range(3):
                jts = [jt for jt in range(NJT) if _grp_interval(g, jt) is not None]
                for h in range(NH):
                    pof = D * (h % 2)
                    ct = h // 2
                    ps_y = {}
                    started = set()
                    for jt in jts:
                        jl = _jl(jt)
                        a, ln = _grp_interval(g, jt)
                        chunks = [(a, min(ln, 512))]
                        if ln > 512:
                            chunks.append((a + 512, ln - 512))
                        for ca, cl in chunks:
                            ps_s = psS.tile([P, 512], F32, name="ps_s", tag="ps_s")
                            nc.tensor.matmul(
                                ps_s[:jl, :cl],
                                kT[pof : pof + D, ct, jt * P : jt * P + jl],
                                qT[pof : pof + D, ct, ca : ca + cl],
                                start=True,
                                stop=True,
                            )
                            pt = phB.tile([P, 512], F16, name="pT", tag="pT", bufs=16)
                            nc.scalar.activation(
                                pt[:jl, :cl],
                                ps_s[:jl, :cl],
                                mybir.ActivationFunctionType.Exp,
                                bias=0.0,
                                scale=SCALE,
                            )
                            mk = _grp_mask(g, jt)
                            if ca == a and mk in ("T1", "T2"):
                                nc.vector.tensor_tensor(
                                    pt[:jl, 0:P],
                                    pt[:jl, 0:P],
                                    maskD[:jl, 0 if mk == "T1" else 1, :],
                                    mybir.AluOpType.mult,
                                )
                            elif ca == a and mk == "TXT":
                                m0 = a - 512
                                nc.vector.tensor_tensor(
                                    pt[:jl, :cl],
                                    pt[:jl, :cl],
                                    maskTx[:jl, m0 : m0 + cl],
                                    mybir.AluOpType.mult,
                                )
                            parts = [(ca, cl, 0)]
                            if ca < 1536 < ca + cl:
                                parts = [
                                    (ca, 1536 - ca, 0),
                                    (1536, ca + cl - 1536, 1536 - ca),
                                ]
                            for pa, pl, poff in parts:
                                ich = _ich_of(pa)
                                off = pa - (0, 512, 1024, 1536)[ich]
                                if ich not in ps_y:
                                    ps_y[ich] = psY.tile(
                                        [D + 1, 512], F32, name=f"ps_y{ich}", tag="ps_y"
                                    )
                                nc.tensor.matmul(
                                    ps_y[ich][:, off : off + pl],
                                    v_ones[:jl, jt, h, :],
                                    pt[:jl, poff : poff + pl],
                                    start=ich not in started,
                                    stop=(jt == jts[-1]),
                                    skip_group_check=True,
                                )
                                started.add(ich)

                    for ich, psy in ps_y.items():
                        i0, ilen = I_CHUNKS[ich]
                        den = phB.tile([1, 512], F16, name="den", tag="den", bufs=4)
                        nc.vector.tensor_copy(den[0:1, :ilen], psy[D : D + 1, :ilen])
                        ps_bc = psS.tile([D, 512], F32, name="ps_bc", tag="ps_s")
                        nc.tensor.matmul(
                            ps_bc[:, :ilen],
                            ones64[0:1, :],
                            den[0:1, :ilen],
                            start=True,
                            stop=True,
                        )
                        rc = phB.tile([D, 512], F32, name="rc", tag="rc", bufs=4)
                        nc.vector.reciprocal_approx_fast(
                            out=rc[:, :ilen], in_=ps_bc[:, :ilen]
                        )
                        nc.vector.tensor_tensor(
                            yT[pof : pof + D, ct, i0 : i0 + ilen],
                            psy[0:D, :ilen],
                            rc[:, :ilen],
                            mybir.AluOpType.mult,
                        )

                # output projection for this group's row tiles
                for it in _GRP_ITS[g]:
                    il = _jl(it)
                    o_sb = phC.tile([P, C], F32, name="o_sb", tag="o_sb")
                    for nch in range(2):
                        ps_o = psS.tile([P, 512], F32, name="ps_o", tag="ps_s")
                        for kt in range(CT):
                            nc.tensor.matmul(
                                ps_o[:il, :384],
                                yT[:, kt, it * P : it * P + il],
                                wp_sb[:, kt, nch * 384 : (nch + 1) * 384],
                                start=(kt == 0),
                                stop=(kt == CT - 1),
                                skip_group_check=True,
                            )
                        nc.any.tensor_copy(
                            o_sb[:il, nch * 384 : (nch + 1) * 384], ps_o[:il, :384]
                        )
                    nc.sync.dma_start(out_d[it * P : it * P + il, :], o_sb[:il, :])

    nc.compile()
    return nc


def _build_mask_np(seg_starts, seg_ends):
    """True = masked. Mirrors reference._build_mask in numpy."""
    ML = 3 * T
    tril = np.tril(np.ones((T, T), dtype=bool))
    sl = np.tril(np.ones((T, T), dtype=bool), -1)
    m = np.zeros((L, L), dtype=bool)
    m[:ML, :ML] = True
    m[0:T, 0:T] = ~tril
    m[T : 2 * T, 0:T] = ~tril
    m[T : 2 * T, T : 2 * T] = ~sl
    m[T : 2 * T, 2 * T : 3 * T] = ~sl
    m[2 * T : 3 * T, 0:T] = ~tril
    m[2 * T : 3 * T, T : 2 * T] = ~tril
    m[2 * T : 3 * T, 2 * T : 3 * T] = ~sl
    m[:ML, ML:] = True
    frames = np.arange(T)[None, :, None]
    allowed = (frames >= seg_starts[:, None, :]) & (frames < seg_ends[:, None, :])
    mask = np.broadcast_to(m[None], (B, L, L)).copy()
    for row0, col_blocks in ((T, (0, 2, 3)), (2 * T, (1, 2, 3))):
        for j in col_blocks:
            c0 = ML + j * N
            mask[:, row0 : row0 + T, c0 : c0 + N] &= ~allowed
    return mask


def get_nc():
    global _NC
    if _NC is None:
        _NC = _build_program()
    return _NC


def make_in_maps(x, Wq, bq, Wk, bk, Wv, bv, Wp, bp, seg_starts, seg_ends):
    mask = _build_mask_np(np.asarray(seg_starts), np.asarray(seg_ends))
    r = np.arange(P)
    maskD = np.empty((P, 2, P), dtype=np.float16)
    maskD[:, 0, :] = (r[:, None] <= r[None, :]).astype(np.float16)  # tril.T
    maskD[:, 1, :] = (r[:, None] < r[None, :]).astype(np.float16)  # strict
    in_maps = []
    for core in range(8):
        b, g = core // 2, core % 2
        gs = slice(g * G, (g + 1) * G)
        allowT = ~mask[b].T  # [j, i]
        maskTx = np.ascontiguousarray(
            allowT[1536:1568, 512:1536].astype(np.float16)
        )
        in_maps.append(
            {
                "xT": np.ascontiguousarray(x[b].T).astype(np.float16),
                "wqT": np.ascontiguousarray(Wq[gs, :].T).astype(np.float16),
                "wkT": np.ascontiguousarray(Wk[gs, :].T).astype(np.float16),
                "wvT": np.ascontiguousarray(Wv[gs, :].T).astype(np.float16),
                "wpT": np.ascontiguousarray(Wp[:, gs].T).astype(np.float16),
                "bqP": np.ascontiguousarray(bq[gs].reshape(CT, P).T),
                "bkP": np.ascontiguousarray(bk[gs].reshape(CT, P).T),
                "bvB": np.broadcast_to(bv[gs], (P, G)).copy(),
                "maskD": maskD,
                "maskTxt": maskTx,
            }
        )
    return in_maps


def kernel(x, Wq, bq, Wk, bk, Wv, bv, Wp, bp, seg_starts, seg_ends, T_motion=None,
           N=None, _trace=False, **_unused):
    x = np.asarray(x, np.float32)
    args = [np.asarray(a, np.float32) for a in (Wq, bq, Wk, bk, Wv, bv, Wp, bp)]
    Wq, bq, Wk, bk, Wv, bv, Wp, bp = args
    nc = get_nc()
    in_maps = make_in_maps(x, Wq, bq, Wk, bk, Wv, bv, Wp, bp, seg_starts, seg_ends)
    res = run_bass_kernel_spmd(nc, in_maps, core_ids=list(range(8)), trace=_trace)
    parts = [r["out_part"] for r in res.results]
    y = np.empty((B, L, C), np.float32)
    for b in range(B):
        y[b] = parts[2 * b] + parts[2 * b + 1] + bp
    if _trace:
        kernel.last_results = res
    return y



# revision 17
# speedup vs baseline: 1.1699x; 1.1699x over previous
"""Cross-conditional GPT2 sparse attention block on 8 Trainium2 NeuronCores.

Sharding: core = (batch b in 0..3) x (head-group g in 0..1, 6 heads each).

v2 schedule: the whole kernel is emitted as one software-pipelined stream so
the PE never stalls (it needs ~3us of continuous execution to reach its max
p-state):
  - unit (g, h) pipeline: scores(i) -> attv(i-1) -> den-bcast(i-2), with
    projection / output-projection matmul chains interleaved as PE filler
    inside the score phases (which are ACT-paced).
  - diag masks applied as ONE strided 3D multiply per (g, h) against a
    precomputed mask stack (instead of one multiply per j-tile).
  - softmax denominator: reciprocal_approx_fast straight off the PSUM row,
    broadcast across the 64 head-dim partitions with an f32r matmul
    (1 cycle/row at N>=512), then a single fused multiply into yT.
  - q/k/v PSUM->SBUF casts (with bias) run on the Scalar engine, freeing DVE.
  - output partials are written f16; host sums pairs + bp (bv folded into bp
    host-side, exact since softmax rows sum to 1).
"""

import sys

sys.path.insert(0, "/opt/trn_rl_repo")

from collections import deque
from contextlib import ExitStack

import numpy as np

import concourse.bacc as bacc
import concourse.bass as bass
import concourse.mybir as mybir
import concourse.tile as tile
from concourse.bass_utils import run_bass_kernel_spmd

# ---- problem constants (hardcoded per spec) ----
B = 4
T = 512
N = 8
C = 768
NHEAD = 12
L = 3 * T + 4 * N  # 1568
P = 128
G = C // 2  # 384 channels per head-group
NH = 6  # heads per core
D = 64  # head dim
ET = C // P  # 6 e-tiles (contraction of x @ W)
CT = G // P  # 3 c-tiles of the group's channels
NJT = (L + P - 1) // P  # 13 j tiles (12x128 + 32)
SLOT = 544  # pt slot width per j-tile (max interval length)
I_CHUNKS = [(0, 512), (512, 512), (1024, 512), (1536, 32)]
ICH0 = (0, 512, 1024, 1536)
SCALE = 1.0 / 8.0  # 1/sqrt(64)

F32 = mybir.dt.float32
F32R = mybir.dt.float32r
BF16 = mybir.dt.bfloat16
F16 = mybir.dt.float16

_NC = None  # cached compiled Bass program


def _jl(jt):
    return P if jt < NJT - 1 else L - (NJT - 1) * P  # 128 or 32


def _ich_of(a):
    return 3 if a == 1536 else a // 512


# (group) -> per-jt score interval (a, ln).
# g0 = upper rows (i 0..512), jts 0..3; g1 = lower rows; g2 = torso+text rows.
def _grp_interval(g, jt):
    j0 = jt * P
    f0 = (jt % 4) * P if jt <= 11 else 0
    if g == 0:
        return (j0, 512 - j0) if jt <= 3 else None
    if g == 1:
        s = j0 if jt <= 3 else f0
        return (512 + s, 512 - s)
    s = j0 if jt <= 3 else f0
    return (1024 + s, 544 - s)


# diag mask kind per (group, jt in 0..11): 'T1' (tril.T) | 'T2' (strict)
def _grp_diag(g, jt):
    if g == 0:
        return "T1"
    if g == 1:
        return "T1" if jt <= 3 else "T2"
    return "T1" if jt <= 7 else "T2"


_GRP_JTS = {0: list(range(0, 4)), 1: list(range(0, 13)), 2: list(range(0, 13))}
_GRP_ITS = {0: range(0, 4), 1: range(4, 8), 2: range(8, 13)}
# i-chunks whose rows belong to group g (for normalization)
_GRP_ICH = {0: [0], 1: [1], 2: [2, 3]}


def _build_program():
    nc = bacc.Bacc("TRN2", target_bir_lowering=False, debug=False)

    xT_d = nc.dram_tensor("xT", [C, L], F16, kind="ExternalInput")
    wq_d = nc.dram_tensor("wqT", [C, G], F16, kind="ExternalInput")
    wk_d = nc.dram_tensor("wkT", [C, G], F16, kind="ExternalInput")
    wv_d = nc.dram_tensor("wvT", [C, G], F16, kind="ExternalInput")
    wp_d = nc.dram_tensor("wpT", [G, C], F16, kind="ExternalInput")
    bq_d = nc.dram_tensor("bqP", [P, CT], F32, kind="ExternalInput")
    bk_d = nc.dram_tensor("bkP", [P, CT], F32, kind="ExternalInput")
    mstk_d = nc.dram_tensor("maskStk", [P, 28 * P], F16, kind="ExternalInput")
    maskt_d = nc.dram_tensor("maskTxt", [32, 1024], F16, kind="ExternalInput")
    out_d = nc.dram_tensor("out_part", [L, C], F16, kind="ExternalOutput")

    # mask-stack slot offset per group (g0: 4 slots, g1: 12, g2: 12)
    MOFF = {0: 0, 1: 4, 2: 16}

    with tile.TileContext(nc) as tc, ExitStack() as big:
        persist = big.enter_context(tc.tile_pool(name="persist", bufs=1))
        phA = big.enter_context(tc.tile_pool(name="phA", bufs=1))
        phB = big.enter_context(tc.tile_pool(name="phB", bufs=1))
        psS = big.enter_context(tc.tile_pool(name="psS", bufs=4, space="PSUM"))
        psY = big.enter_context(tc.tile_pool(name="psY", bufs=4, space="PSUM"))

        # persistent SBUF tensors
        qT = persist.tile([P, CT, L], F16, name="qT")
        kT = persist.tile([P, CT, L], F16, name="kT")
        v_ones = persist.tile([P, NJT, NH, D + 1], F16, name="v_ones")
        maskStk = persist.tile([P, 28, P], F16, name="maskStk_sb")
        maskTx = persist.tile([32, 1024], F16, name="maskTx_sb")
        yT = persist.tile([P, CT, L], F16, name="yT")
        wp_sb = persist.tile([P, CT, C], F16, name="wp_sb")

        nc.sync.dma_start(maskStk[:], mstk_d.rearrange("p (s c) -> p s c", c=P))
        nc.sync.dma_start(maskTx[:], maskt_d[:])
        nc.sync.dma_start(wp_sb[:], wp_d.rearrange("(ct p) n -> p ct n", p=P))
        nc.gpsimd.memset(v_ones[:], 1.0)

        # ---------- Phase A tiles + input DMA ----------
        xT = phA.tile([P, ET, L], F16, name="xT_sb")
        wq_sb = phA.tile([P, ET, G], F16, name="wq_sb")
        wk_sb = phA.tile([P, ET, G], F16, name="wk_sb")
        wv_sb = phA.tile([P, ET, G], F16, name="wv_sb")
        bq_sb = phA.tile([P, CT], F32, name="bq_sb")
        bk_sb = phA.tile([P, CT], F32, name="bk_sb")

        for et in range(ET):
            nc.sync.dma_start(xT[:, et, :], xT_d[et * P : (et + 1) * P, :])
        for w_sb, w_d in ((wq_sb, wq_d), (wk_sb, wk_d), (wv_sb, wv_d)):
            for et in range(ET):
                nc.sync.dma_start(w_sb[:, et, :], w_d[et * P : (et + 1) * P, :])
        nc.sync.dma_start(bq_sb[:], bq_d[:])
        nc.sync.dma_start(bk_sb[:], bk_d[:])

        # ---------- projection chain emitters (PE + ACT cast) ----------
        def emit_qk_chain(dst, w_sb, b_sb, ct, ich):
            i0, ilen = I_CHUNKS[ich]
            ps = psS.tile([P, 512], F32, name="ps_qk", tag="ps_s")
            for et in range(ET):
                nc.tensor.matmul(
                    ps[:, :ilen],
                    w_sb[:, et, ct * P : (ct + 1) * P],
                    xT[:, et, i0 : i0 + ilen],
                    start=(et == 0),
                    stop=(et == ET - 1),
                    skip_group_check=True,
                )
            nc.vector.tensor_scalar(
                dst[:, ct, i0 : i0 + ilen],
                ps[:, :ilen],
                b_sb[:, ct : ct + 1],
                None,
                mybir.AluOpType.add,
            )

        def emit_v_chain(it):
            il = _jl(it)
            ps = psS.tile([P, 512], F32, name="ps_v", tag="ps_s")
            for et in range(ET):
                nc.tensor.matmul(
                    ps[:il, :G],
                    xT[:, et, it * P : it * P + il],
                    wv_sb[:, et, :],
                    start=(et == 0),
                    stop=(et == ET - 1),
                    skip_group_check=True,
                )
            nc.vector.tensor_copy(
                v_ones[:il, it, :, 0:D],
                ps[:il, :G].rearrange("p (h d) -> p h d", h=NH),
            )

        def emit_outproj_chain(it, nch):
            il = _jl(it)
            ps_o = psS.tile([P, 512], F32, name="ps_o", tag="ps_s")
            for kt in range(CT):
                nc.tensor.matmul(
                    ps_o[:il, :G],
                    yT[:, kt, it * P : it * P + il],
                    wp_sb[:, kt, nch * G : (nch + 1) * G],
                    start=(kt == 0),
                    stop=(kt == CT - 1),
                    skip_group_check=True,
                )
            o_sb = phB.tile([P, G], F16, name="o_sb", tag="o_sb", bufs=3)
            nc.vector.tensor_copy(o_sb[:il, :], ps_o[:il, :G])
            nc.sync.dma_start(
                out_d[it * P : it * P + il, nch * G : (nch + 1) * G], o_sb[:il, :]
            )

        # ---------- attention unit emitters ----------
        units = [(g, h) for g in range(3) for h in range(NH)]
        urec = [dict() for _ in units]  # per-unit state (pt tile, chunks, psy)

        fillers = deque()

        def pop_filler(n=1):
            for _ in range(n):
                if fillers:
                    fillers.popleft()()

        def emit_scores(i):
            g, h = units[i]
            pof = D * (h % 2)
            ct = h // 2
            pt = phB.tile([P, NJT, SLOT], F16, name="pt", tag="pt", bufs=3)
            chunks = []  # (jt, slot, ca, cl, slot_off)
            cols = 0
            for slot, jt in enumerate(_GRP_JTS[g]):
                iv = _grp_interval(g, jt)
                jl = _jl(jt)
                a, ln = iv
                cparts = [(a, min(ln, 512))]
                if ln > 512:
                    cparts.append((a + 512, ln - 512))
                for ca, cl in cparts:
                    ps_s = psS.tile([P, 512], F32, name="ps_s", tag="ps_s")
                    nc.tensor.matmul(
                        ps_s[:jl, :cl],
                        kT[pof : pof + D, ct, jt * P : jt * P + jl],
                        qT[pof : pof + D, ct, ca : ca + cl],
                        start=True,
                        stop=True,
                        skip_group_check=True,
                    )
                    nc.scalar.activation(
                        pt[:jl, slot, ca - a : ca - a + cl],
                        ps_s[:jl, :cl],
                        mybir.ActivationFunctionType.Exp,
                        bias=0.0,
                        scale=SCALE,
                    )
                    chunks.append((jt, slot, ca, cl, ca - a))
                    cols += cl
                    if cols >= 1024:
                        cols -= 1024
                        pop_filler()
            urec[i]["pt"] = pt
            urec[i]["chunks"] = chunks

        def emit_mask(i):
            g, h = units[i]
            pt = urec[i]["pt"]
            ndiag = 4 if g == 0 else 12
            nc.vector.tensor_tensor(
                pt[:, 0:ndiag, 0:P],
                pt[:, 0:ndiag, 0:P],
                maskStk[:, MOFF[g] : MOFF[g] + ndiag, :],
                mybir.AluOpType.mult,
            )
            if g >= 1:
                m0 = 0 if g == 1 else 512
                nc.vector.tensor_tensor(
                    pt[0:32, 12, 0:512],
                    pt[0:32, 12, 0:512],
                    maskTx[0:32, m0 : m0 + 512],
                    mybir.AluOpType.mult,
                )

        def emit_attv(i):
            g, h = units[i]
            pt = urec[i]["pt"]
            ps_y = {}
            started = set()
            last_jt = _GRP_JTS[g][-1]
            for jt, slot, ca, cl, soff in urec[i]["chunks"]:
                jl = _jl(jt)
                parts = [(ca, cl, soff)]
                if ca < 1536 < ca + cl:
                    parts = [
                        (ca, 1536 - ca, soff),
                        (1536, ca + cl - 1536, soff + 1536 - ca),
                    ]
                for pa, pl, poff in parts:
                    ich = _ich_of(pa)
                    off = pa - ICH0[ich]
                    if ich not in ps_y:
                        ps_y[ich] = psY.tile(
                            [D + 1, 512], F32, name=f"ps_y{ich}", tag="ps_y"
                        )
                    nc.tensor.matmul(
                        ps_y[ich][:, off : off + pl],
                        v_ones[:jl, jt, h, :],
                        pt[:jl, slot, poff : poff + pl],
                        start=ich not in started,
                        stop=(jt == last_jt),
                        skip_group_check=True,
                    )
                    started.add(ich)
            urec[i]["ps_y"] = ps_y

        def emit_den_copy(i):
            # pull the denominator row (PSUM partition 64) into SBUF partition
            # 0 on the (idle) GpSimd engine; custom DVE ops can't read a
            # nonzero base partition and PSUM APs must be 32-aligned.
            dens = []
            for ich, psy in urec[i]["ps_y"].items():
                ilen = I_CHUNKS[ich][1]
                den = phB.tile([1, 512], F32, name="den", tag="den", bufs=4)
                nc.vector.tensor_copy(den[0:1, :ilen], psy[D : D + 1, :ilen])
                dens.append((ich, psy, den))
            urec[i]["dens"] = dens

        def emit_norm_recip(i):
            # reciprocal on DVE, then broadcast across the 64 head-dim
            # partitions on GpSimd. The consuming multiply runs later in the
            # same loop so GpSimd latency never stalls DVE/PE.
            recs = []
            for ich, psy, den in urec[i]["dens"]:
                ilen = I_CHUNKS[ich][1]
                rc = phB.tile([1, 512], F32, name="rc", tag="rc", bufs=4)
                nc.vector.reciprocal_approx_fast(
                    out=rc[0:1, :ilen], in_=den[0:1, :ilen]
                )
                rc_bc = phB.tile([D, 512], F32, name="rc_bc", tag="rc_bc", bufs=4)
                nc.gpsimd.partition_broadcast(rc_bc[:, :ilen], rc[0:1, :ilen])
                recs.append((ich, psy, rc_bc))
            urec[i]["recs"] = recs

        def emit_norm_mult(i):
            g, h = units[i]
            pof = D * (h % 2)
            ct = h // 2
            for ich, psy, rc_bc in urec[i]["recs"]:
                i0, ilen = I_CHUNKS[ich]
                nc.vector.tensor_tensor(
                    yT[pof : pof + D, ct, i0 : i0 + ilen],
                    psy[0:D, :ilen],
                    rc_bc[:, :ilen],
                    mybir.AluOpType.mult,
                )

        # ---------- upfront: phase A for g0's needs ----------
        for ct in range(CT):
            emit_qk_chain(kT, wk_sb, bk_sb, ct, 0)
            emit_qk_chain(qT, wq_sb, bq_sb, ct, 0)
        for it in range(4):
            emit_v_chain(it)

        # filler groups consumed during g0 / g1 / g2
        # g1/g2 attend ALL key tiles, so every kT chunk and every v tile must
        # be emitted before g1's first score/attv; only qT splits by row group.
        fill_g0 = []
        for ich in (1, 2, 3):
            for ct in range(CT):
                fill_g0.append(
                    lambda ct=ct, ich=ich: emit_qk_chain(kT, wk_sb, bk_sb, ct, ich)
                )
        for ct in range(CT):
            fill_g0.append(lambda ct=ct: emit_qk_chain(qT, wq_sb, bq_sb, ct, 1))
        for it in range(4, NJT):
            fill_g0.append(lambda it=it: emit_v_chain(it))

        fill_g1 = []
        for ich in (2, 3):
            for ct in range(CT):
                fill_g1.append(
                    lambda ct=ct, ich=ich: emit_qk_chain(qT, wq_sb, bq_sb, ct, ich)
                )

        FILL = {0: fill_g0, 1: fill_g1, 2: []}

        # ---------- main software-pipelined loop ----------
        prev_g = None
        for i, (g, h) in enumerate(units):
            if g != prev_g:
                # drain leftovers of the previous phase's fillers (their
                # outputs gate this group's first scores), then load new ones
                while fillers:
                    fillers.popleft()()
                fillers.extend(FILL[g])
                prev_g = g
                if g == 2:
                    # out-projection of g0 becomes filler now (its yT rows
                    # finished at norm_mult(g0, h5) which was emitted at i-3)
                    for it in _GRP_ITS[0]:
                        for nch in range(2):
                            fillers.append(
                                lambda it=it, nch=nch: emit_outproj_chain(it, nch)
                            )
            if i == 15:
                # g1's yT finished at norm_mult(unit 11) emitted at i=14
                for it in _GRP_ITS[1]:
                    for nch in range(2):
                        fillers.append(
                            lambda it=it, nch=nch: emit_outproj_chain(it, nch)
                        )
            if i >= 2:
                emit_norm_recip(i - 2)
            emit_scores(i)
            emit_mask(i)
            if i >= 1:
                emit_attv(i - 1)
                emit_den_copy(i - 1)
            if i >= 2:
                emit_norm_mult(i - 2)
            pop_filler()

        # ---------- tail ----------
        nunits = len(units)
        emit_attv(nunits - 1)
        emit_den_copy(nunits - 1)
        for i in (nunits - 2, nunits - 1):
            emit_norm_recip(i)
            emit_norm_mult(i)
        while fillers:
            fillers.popleft()()
        for it in _GRP_ITS[2]:
            for nch in range(2):
                emit_outproj_chain(it, nch)

    nc.compile()
    return nc


def _build_mask_np(seg_starts, seg_ends):
    """True = masked. Mirrors reference._build_mask in numpy."""
    ML = 3 * T
    tril = np.tril(np.ones((T, T), dtype=bool))
    sl = np.tril(np.ones((T, T), dtype=bool), -1)
    m = np.zeros((L, L), dtype=bool)
    m[:ML, :ML] = True
    m[0:T, 0:T] = ~tril
    m[T : 2 * T, 0:T] = ~tril
    m[T : 2 * T, T : 2 * T] = ~sl
    m[T : 2 * T, 2 * T : 3 * T] = ~sl
    m[2 * T : 3 * T, 0:T] = ~tril
    m[2 * T : 3 * T, T : 2 * T] = ~tril
    m[2 * T : 3 * T, 2 * T : 3 * T] = ~sl
    m[:ML, ML:] = True
    frames = np.arange(T)[None, :, None]
    allowed = (frames >= seg_starts[:, None, :]) & (frames < seg_ends[:, None, :])
    mask = np.broadcast_to(m[None], (B, L, L)).copy()
    for row0, col_blocks in ((T, (0, 2, 3)), (2 * T, (1, 2, 3))):
        for j in col_blocks:
            c0 = ML + j * N
            mask[:, row0 : row0 + T, c0 : c0 + N] &= ~allowed
    return mask


def get_nc():
    global _NC
    if _NC is None:
        _NC = _build_program()
    return _NC


def _build_maskstk():
    r = np.arange(P)
    t1 = (r[:, None] <= r[None, :]).astype(np.float16)  # tril.T
    t2 = (r[:, None] < r[None, :]).astype(np.float16)  # strict
    stk = np.empty((P, 28, P), dtype=np.float16)
    s = 0
    for g in range(3):
        for jt in _GRP_JTS[g][: (4 if g == 0 else 12)]:
            stk[:, s, :] = t1 if _grp_diag(g, jt) == "T1" else t2
            s += 1
    assert s == 28
    return stk.reshape(P, 28 * P)


def make_in_maps(x, Wq, bq, Wk, bk, Wv, bv, Wp, bp, seg_starts, seg_ends):
    mask = _build_mask_np(np.asarray(seg_starts), np.asarray(seg_ends))
    maskstk = _build_maskstk()
    in_maps = []
    for core in range(8):
        b, g = core // 2, core % 2
        gs = slice(g * G, (g + 1) * G)
        allowT = ~mask[b].T  # [j, i]
        maskTx = np.ascontiguousarray(
            allowT[1536:1568, 512:1536].astype(np.float16)
        )
        in_maps.append(
            {
                "xT": np.ascontiguousarray(x[b].T).astype(np.float16),
                "wqT": np.ascontiguousarray(Wq[gs, :].T).astype(np.float16),
                "wkT": np.ascontiguousarray(Wk[gs, :].T).astype(np.float16),
                "wvT": np.ascontiguousarray(Wv[gs, :].T).astype(np.float16),
                "wpT": np.ascontiguousarray(Wp[:, gs].T).astype(np.float16),
                "bqP": np.ascontiguousarray(bq[gs].reshape(CT, P).T),
                "bkP": np.ascontiguousarray(bk[gs].reshape(CT, P).T),
                "maskStk": maskstk,
                "maskTxt": maskTx,
            }
        )
    return in_maps


def kernel(x, Wq, bq, Wk, bk, Wv, bv, Wp, bp, seg_starts, seg_ends, T_motion=None,
           N=None, _trace=False, **_unused):
    x = np.asarray(x, np.float32)
    args = [np.asarray(a, np.float32) for a in (Wq, bq, Wk, bk, Wv, bv, Wp, bp)]
    Wq, bq, Wk, bk, Wv, bv, Wp, bp = args
    nc = get_nc()
    in_maps = make_in_maps(x, Wq, bq, Wk, bk, Wv, bv, Wp, bp, seg_starts, seg_ends)
    res = run_bass_kernel_spmd(nc, in_maps, core_ids=list(range(8)), trace=_trace)
    parts = [np.asarray(r["out_part"], np.float32) for r in res.results]
    # v-bias folds into the output bias exactly: att rows sum to 1, so
    # y = att@(v+bv) = att@v + bv, and (y+bv)@Wp.T = y@Wp.T + bv@Wp.T
    bp_eff = bp + bv @ Wp.T
    y = np.empty((B, L, C), np.float32)
    for b in range(B):
        y[b] = parts[2 * b] + parts[2 * b + 1] + bp_eff
    if _trace:
        kernel.last_results = res
    return y
